# revision 9
# baseline (speedup 1.0000x reference)
"""CAM (channel self-attention) kernel for Trainium2 — 8 NeuronCores, batch-parallel.

Math per batch element b (A = x[b] reshaped [N=4096, C=512]):
    G = A^T A                  [C, C]   (symmetric!)
    P = softmax_rows(G)        [C, C]
    Y = A P                    [N, C]
    out = gamma * Y + x

Sharding: data-parallel over batch — core i handles batch element i.
No cross-core communication needed.

Schedule (v5): the kernel is HBM-bound on both the 8 MiB input read and
the 8 MiB output write (358 GB/s/core), with a hard dependency between
them (P needs all of A).  Compute is balanced across the two DMA
windows so the PE hides under both.  fp8 is used ONLY where the PE
consumes it (DoubleRow Y matmuls, ~1.5x bf16 at FD=512) and ONLY
produced by ACT — DVE/GpSimd fp8 stores run ~6x slow, and concurrent
GpSimd tensor work contends DVE's SBUF ports (both measured on HW).
  - Input phase (Sync + GpSimd-SWDGE rings, alternating groups): per
    128-row chunk k, DVE cast -> A16 (bf16); 2 PE bf16 transposes
    (ci 0,1 only) -> PSUM -> strided ACT copy into AT8[:, 0:2] (casts
    to fp8e4); upper-triangle bf16 Gram matmuls (free dims
    512/384/256/128 exploiting G's symmetry) into 3 PSUM banks.
    8 warm-up matmuls (into the g0 bank, reset later by start=True)
    spin the PE HAM clock gate up (1.2 -> 2.4 GHz) during the DMA
    preamble window.
  - Lower triangle of G via 6 PE f32 transposes of the upper blocks.
  - softmax: DVE row-max (negated) -> ACT exp (bf16 E) with fused
    row-sum -> DVE reciprocal -> DVE scale -> P16 -> ACT cast -> P8.
  - Output phase (ACT + Sync rings, alternating groups): per row chunk
    t, the ci 2,3 transposes of source chunk t+3 are emitted first
    (lookahead keeps AT8[:, 2:4] three chunks ahead of the consuming
    matmul; the first three are primed during the softmax window, when
    the PE is otherwise idle).  Y = A P via 2 fp8 DoubleRow matmuls
    (lhsT = AT8, rhs = P8, contraction 256/instr); DVE
    scalar_tensor_tensor epilogue out = (Y * gamma) + A32 in f32
    (exact x-residual), staged 512 KiB groups.
fp8 quantization only touches the gamma*Y term; tolerance is rel 2e-2.
"""

import numpy as np

import concourse.tile as tile
from concourse import bacc, mybir
from concourse.bass_utils import run_bass_kernel_spmd
from concourse.masks import make_identity

B = 8
H = 64
W = 64
C = 512
HW = H * W            # 4096 rows per batch element
NT = HW // 128        # 32 row chunks of 128
CT = C // 128         # 4 col chunks of 128
GRP = 4               # row chunks per input DMA group (1 MiB)
OGRP = 2              # row chunks per output DMA group (512 KiB)
LOOK = 3              # transpose lookahead (output chunks) in phase 2

F32 = mybir.dt.float32
BF16 = mybir.dt.bfloat16
FP8 = mybir.dt.float8e4
DR = mybir.MatmulPerfMode.DoubleRow

_CACHE = {}


def _emit(nc, tc, out, x, gamma):
    from contextlib import ExitStack

    with ExitStack() as ctx:
        big = ctx.enter_context(tc.tile_pool(name="big", bufs=1))
        small = ctx.enter_context(tc.tile_pool(name="small", bufs=1))
        stat = ctx.enter_context(tc.tile_pool(name="stat", bufs=4))
        ostage = ctx.enter_context(tc.tile_pool(name="ostage", bufs=4))
        gps = ctx.enter_context(tc.tile_pool(name="gps", bufs=1, space="PSUM"))
        wps = ctx.enter_context(tc.tile_pool(name="wps", bufs=5, space="PSUM"))

        A32 = big.tile([128, NT, C], F32)     # x rows, n on partitions (exact)
        A16 = big.tile([128, NT, C], BF16)    # bf16 cast of A32
        AT8 = big.tile([128, CT, HW], FP8)    # A^T, c on partitions
        G32 = big.tile([128, CT, C], F32)     # full Gram matrix in SBUF
        E16 = big.tile([128, CT, C], BF16)    # exp(G - rowmax)
        P16 = big.tile([128, CT, C], BF16)    # softmax(G) in bf16
        P8 = big.tile([128, CT, C], FP8)      # softmax(G) in fp8

        # Upper-triangle Gram accumulators: G[mi-chunk, mi*128:].
        # g1 (384 cols) and g3 (128 cols) share one PSUM bank.
        g0 = gps.tile([128, C], F32, name="g0", tag="g0")
        g13 = gps.tile([128, C], F32, name="g13", tag="g13")
        g2 = gps.tile([128, C - 256], F32, name="g2", tag="g2")
        g_ps = [g0[:], g13[:, 0:384], g2[:], g13[:, 384:512]]

        # PE warm-up first (needs no identity): the HAM clock gate holds the
        # PE at 1.2 GHz until it has been busy ~3.4us; burn that during the
        # DMA preamble window.  Warm matmuls write into the g0 bank, which
        # the first real Gram matmul resets via start=True.
        warm_src = small.tile([128, C], FP8)
        nc.gpsimd.memset(warm_src[:], 0.0)
        for wi in range(8):
            nc.tensor.matmul(
                g0[:], warm_src[:, 0:128], warm_src[:],
                start=(wi == 0), stop=(wi == 7),
            )

        ident = small.tile([128, 128], BF16)
        make_identity(nc, ident[:])
        ident32 = small.tile([128, 128], F32)
        make_identity(nc, ident32[:])

        gB = small.tile([128, 1], F32)        # gamma broadcast to all partitions

        # First loads chunk-granular so the PE can start early, then 1 MiB.
        # Groups alternate between the Sync HWDGE ring and the GpSimd SWDGE
        # ring so one ring's completion bubble hides under the other.
        load_groups = [1, 1, 2] + [GRP] * ((NT - 4) // GRP)
        assert sum(load_groups) == NT
        k0 = 0
        for gi, gsz in enumerate(load_groups):
            r0 = k0 * 128
            r1 = (k0 + gsz) * 128
            ieng = nc.sync if gi % 2 == 0 else nc.gpsimd
            ieng.dma_start(
                A32[:, k0:k0 + gsz, :],
                x[r0:r1, :].rearrange("(t p) c -> p t c", p=128),
            )
            if gi == 0:
                # gamma: tiny load on the ACT HWDGE ring, off the input path
                nc.scalar.dma_start(gB[:], gamma[:])
            for j in range(gsz):
                k = k0 + j
                # cast f32 -> bf16 (DVE)
                nc.vector.tensor_copy(A16[:, k, :], A32[:, k, :])
                # A^T blocks ci 0,1 -> PSUM -> one strided fp8 copy (ACT);
                # ci 2,3 are deferred to phase 2 to balance PE load
                tp = wps.tile([128, 2 * 128], BF16, name="tp", tag="w")
                for ci in range(2):
                    nc.tensor.transpose(
                        tp[:, ci * 128:(ci + 1) * 128],
                        A16[:, k, ci * 128:(ci + 1) * 128],
                        ident[:],
                    )
                nc.scalar.copy(
                    AT8[:, 0:2, k * 128:(k + 1) * 128],
                    tp[:].rearrange("p (ci n) -> p ci n", ci=2),
                )
                # upper-triangle Gram matmuls for this chunk (bf16)
                for mi in range(CT):
                    nc.tensor.matmul(
                        g_ps[mi],
                        A16[:, k, mi * 128:(mi + 1) * 128],
                        A16[:, k, mi * 128:],
                        start=(k == 0),
                        stop=(k == NT - 1),
                        # g1/g3 share a bank; per-element has_written makes
                        # disjoint-region groups safe on HW
                        skip_group_check=(mi % 2 == 1),
                    )
            k0 += gsz

        # G (upper) PSUM -> SBUF
        for mi in range(CT):
            if mi % 2 == 0:
                nc.vector.tensor_copy(G32[:, mi, mi * 128:], g_ps[mi])
            else:
                nc.scalar.copy(G32[:, mi, mi * 128:], g_ps[mi])
        # reconstruct lower triangle: G[mi, j] = G[j, mi]^T for j < mi
        for mi in range(1, CT):
            for j in range(mi):
                lb = wps.tile([128, 128], F32, name="lb", tag="w")
                nc.tensor.transpose(
                    lb[:], G32[:, j, mi * 128:(mi + 1) * 128], ident32[:])
                if (mi + j) % 2 == 0:
                    nc.vector.tensor_copy(G32[:, mi, j * 128:(j + 1) * 128], lb[:])
                else:
                    nc.scalar.copy(G32[:, mi, j * 128:(j + 1) * 128], lb[:])

        # softmax over rows of G (free axis); P8 needs an ACT pass (the only
        # engine with fast fp8 stores), interleaved chunk by chunk
        for mi in range(CT):
            nmax = stat.tile([128, 1], F32)
            nc.vector.tensor_reduce(
                nmax[:], G32[:, mi, :],
                axis=mybir.AxisListType.X, op=mybir.AluOpType.max, negate=True,
            )
            esum = stat.tile([128, 1], F32)
            nc.scalar.activation(
                E16[:, mi, :], G32[:, mi, :],
                mybir.ActivationFunctionType.Exp,
                bias=nmax[:], scale=1.0, accum_out=esum[:],
            )
            rsum = stat.tile([128, 1], F32)
            nc.vector.reciprocal(rsum[:], esum[:])
            nc.vector.tensor_scalar_mul(P16[:, mi, :], E16[:, mi, :], rsum[:])
            nc.scalar.copy(P8[:, mi, :], P16[:, mi, :])

        # deferred A^T blocks ci 2,3 of source chunk k -> AT8[:, 2:4]
        def emit_t23(k):
            tp2 = wps.tile([128, 2 * 128], BF16, name="tp2", tag="w")
            for ci in (2, 3):
                nc.tensor.transpose(
                    tp2[:, (ci - 2) * 128:(ci - 1) * 128],
                    A16[:, k, ci * 128:(ci + 1) * 128],
                    ident[:],
                )
            nc.scalar.copy(
                AT8[:, 2:4, k * 128:(k + 1) * 128],
                tp2[:].rearrange("p (ci n) -> p ci n", ci=2),
            )

        # prime the lookahead window during the softmax (PE idle there)
        for k in range(min(LOOK, NT)):
            emit_t23(k)

        # Y = A @ P (fp8 DoubleRow), epilogue out = gamma * Y + x (f32, exact)
        out_groups = [OGRP] * (NT // OGRP - 1) + [1, 1]
        t0 = 0
        for h, osz in enumerate(out_groups):
            r0 = t0 * 128
            r1 = (t0 + osz) * 128
            o32 = ostage.tile([128, OGRP, C], F32)
            for j in range(osz):
                t = t0 + j
                if t + LOOK < NT:
                    emit_t23(t + LOOK)
                y = wps.tile([128, C], F32, name="y", tag="w")
                for cp in range(CT // 2):
                    nc.tensor.matmul(
                        y[:],
                        AT8[:, 2 * cp:2 * cp + 2, t * 128:(t + 1) * 128],
                        P8[:, 2 * cp:2 * cp + 2, :],
                        start=(cp == 0),
                        stop=(cp == CT // 2 - 1),
                        perf_mode=DR,
                    )
                nc.vector.scalar_tensor_tensor(
                    o32[:, j, :], y[:], gB[:], A32[:, t, :],
                    op0=mybir.AluOpType.mult, op1=mybir.AluOpType.add,
                )
            # alternate output rings: ACT HWDGE and (now idle) Sync HWDGE
            oeng = nc.scalar if h % 2 == 0 else nc.sync
            oeng.dma_start(
                out[r0:r1, :].rearrange("(t p) c -> p t c", p=128),
                o32[:, 0:osz, :],
            )
            t0 += osz


def build():
    nc = bacc.Bacc("TRN2", target_bir_lowering=False, debug=False)
    x = nc.dram_tensor("x", [HW, C], F32, kind="ExternalInput").ap()
    gamma = nc.dram_tensor("gamma", [128, 1], F32, kind="ExternalInput").ap()
    out = nc.dram_tensor("out", [HW, C], F32, kind="ExternalOutput").ap()
    with tile.TileContext(nc) as tc:
        _emit(nc, tc, out, x, gamma)
    nc.compile()
    return nc


def kernel(x: np.ndarray, gamma: np.ndarray, trace: bool = False):
    assert x.shape == (B, H, W, C), x.shape
    if "nc" not in _CACHE:
        _CACHE["nc"] = build()
    nc = _CACHE["nc"]

    g128 = np.full((128, 1), np.float32(np.asarray(gamma).reshape(-1)[0]),
                   dtype=np.float32)
    in_maps = [
        {
            "x": np.ascontiguousarray(
                np.asarray(x[i], dtype=np.float32).reshape(HW, C)),
            "gamma": g128,
        }
        for i in range(B)
    ]
    if trace:
        res = run_bass_kernel_spmd(nc, in_maps, core_ids=list(range(B)),
                                   trace=True)
    else:
        # Force-untraced: a stray BASS_TRACE in the environment would route
        # through profiling hooks this image may not have.
        import os
        prev = os.environ.get("BASS_NEVER_TRACE")
        os.environ["BASS_NEVER_TRACE"] = "1"
        try:
            res = run_bass_kernel_spmd(nc, in_maps, core_ids=list(range(B)))
        finally:
            if prev is None:
                os.environ.pop("BASS_NEVER_TRACE", None)
            else:
                os.environ["BASS_NEVER_TRACE"] = prev
    _CACHE["last_result"] = res
    out = np.stack([res.results[i]["out"] for i in range(B)], axis=0)
    return out.reshape(B, H, W, C).astype(np.float32)


# revision 16
# speedup vs baseline: 1.0304x; 1.0304x over previous
"""CAM (channel self-attention) kernel for Trainium2 — 8 NeuronCores, batch-parallel.

Math per batch element b (A = x[b] reshaped [N=4096, C=512]):
    G = A^T A                  [C, C]   (symmetric!)
    P = softmax_rows(G)        [C, C]
    Y = A P                    [N, C]
    out = gamma * Y + x

Sharding: data-parallel over batch — core i handles batch element i.
No cross-core communication needed.

Schedule (v6): the kernel is HBM-bound on both the 8 MiB input read and
the 8 MiB output write (358 GB/s/core), with a hard dependency between
them (P needs all of A).  fp8 is used ONLY where the PE consumes it
(DoubleRow Y matmuls, ~1.5x bf16 at FD=512) and ONLY produced by ACT —
DVE/GpSimd fp8 stores run ~6x slow, concurrent GpSimd tensor work
contends DVE's SBUF ports, and GpSimd SWDGE DMAs slow PE weight loads
(all measured on HW).
  - Input phase (Sync HWDGE ring): per 128-row chunk k, DVE cast ->
    A16 (bf16); 4 PE bf16 transposes -> PSUM -> one contiguous ACT
    copy into AT8 (k-major layout, casts to fp8e4); upper-triangle
    bf16 Gram matmuls (free dims 512/384/256/128 exploiting G's
    symmetry) into 3 PSUM banks.  8 warm-up matmuls (into the g0 bank,
    reset later by start=True) spin the PE HAM clock gate up
    (1.2 -> 2.4 GHz) during the DMA preamble window.
  - Lower triangle of G via 6 PE f32 transposes of the upper blocks.
  - softmax: DVE row-max (negated) -> ACT exp (bf16 E) with fused
    row-sum -> DVE reciprocal -> DVE scale -> P16 -> ACT cast -> P8,
    chunk-interleaved so phase 2 can start after the second cast.
  - Output phase (ACT + Sync rings, alternating groups): Y = A P via
    2 fp8 DoubleRow matmuls per row chunk (lhsT = AT8[:, t] Ko-pairs,
    rhs = P8, contraction 256/instr); DVE scalar_tensor_tensor
    epilogue out = (Y * gamma) + A32 in f32 (exact x-residual), staged
    512 KiB groups.
fp8 quantization only touches the gamma*Y term; tolerance is rel 2e-2.
"""

import numpy as np

import concourse.tile as tile
from concourse import bacc, mybir
from concourse.bass_utils import run_bass_kernel_spmd
from concourse.masks import make_identity

B = 8
H = 64
W = 64
C = 512
HW = H * W            # 4096 rows per batch element
NT = HW // 128        # 32 row chunks of 128
CT = C // 128         # 4 col chunks of 128
GRP = 4               # row chunks per input DMA group (1 MiB)
OGRP = 2              # row chunks per output DMA group (512 KiB)

F32 = mybir.dt.float32
BF16 = mybir.dt.bfloat16
FP8 = mybir.dt.float8e4
DR = mybir.MatmulPerfMode.DoubleRow

_CACHE = {}


def _emit(nc, tc, out, x, gamma):
    from contextlib import ExitStack

    with ExitStack() as ctx:
        big = ctx.enter_context(tc.tile_pool(name="big", bufs=1))
        small = ctx.enter_context(tc.tile_pool(name="small", bufs=1))
        stat = ctx.enter_context(tc.tile_pool(name="stat", bufs=4))
        ostage = ctx.enter_context(tc.tile_pool(name="ostage", bufs=4))
        gps = ctx.enter_context(tc.tile_pool(name="gps", bufs=1, space="PSUM"))
        wps = ctx.enter_context(tc.tile_pool(name="wps", bufs=5, space="PSUM"))

        A32 = big.tile([128, NT, C], F32)     # x rows, n on partitions (exact)
        A16 = big.tile([128, NT, C], BF16)    # bf16 cast of A32
        # A^T in k-major layout: AT8[p, k, ci*128 + n] = A[k*128+n, ci*128+p];
        # phase-2 lhsT slices are [p, 2, 128] with Ko step 128 B (%16 == 0)
        AT8 = big.tile([128, NT, C], FP8)
        G32 = big.tile([128, CT, C], F32)     # full Gram matrix in SBUF
        E16 = big.tile([128, CT, C], BF16)    # exp(G - rowmax)
        P16 = big.tile([128, CT, C], BF16)    # softmax(G) in bf16
        P8 = big.tile([128, CT, C], FP8)      # softmax(G) in fp8

        # Upper-triangle Gram accumulators: G[mi-chunk, mi*128:].
        # g1 (384 cols) and g3 (128 cols) share one PSUM bank.
        g0 = gps.tile([128, C], F32, name="g0", tag="g0")
        g13 = gps.tile([128, C], F32, name="g13", tag="g13")
        g2 = gps.tile([128, C - 256], F32, name="g2", tag="g2")
        g_ps = [g0[:], g13[:, 0:384], g2[:], g13[:, 384:512]]

        # PE warm-up first (needs no identity): the HAM clock gate holds the
        # PE at 1.2 GHz until it has been busy ~3.4us; burn that during the
        # DMA preamble window.  Warm matmuls write into the g0 bank, which
        # the first real Gram matmul resets via start=True.
        warm_src = small.tile([128, C], FP8)
        nc.gpsimd.memset(warm_src[:], 0.0)
        for wi in range(8):
            nc.tensor.matmul(
                g0[:], warm_src[:, 0:128], warm_src[:],
                start=(wi == 0), stop=(wi == 7),
            )

        ident = small.tile([128, 128], BF16)
        make_identity(nc, ident[:])
        ident32 = small.tile([128, 128], F32)
        make_identity(nc, ident32[:])

        gB = small.tile([128, 1], F32)        # gamma broadcast to all partitions

        # First loads chunk-granular so the PE can start early, then 1 MiB.
        # Groups alternate between the Sync HWDGE ring and the GpSimd SWDGE
        # ring so one ring's completion bubble hides under the other.
        load_groups = [1, 1, 2] + [GRP] * ((NT - 4) // GRP)
        assert sum(load_groups) == NT
        k0 = 0
        for gi, gsz in enumerate(load_groups):
            r0 = k0 * 128
            r1 = (k0 + gsz) * 128
            nc.sync.dma_start(
                A32[:, k0:k0 + gsz, :],
                x[r0:r1, :].rearrange("(t p) c -> p t c", p=128),
            )
            if gi == 0:
                # gamma: tiny load on the ACT HWDGE ring, off the input path
                nc.scalar.dma_start(gB[:], gamma[:])
            for j in range(gsz):
                k = k0 + j
                # cast f32 -> bf16 (DVE)
                nc.vector.tensor_copy(A16[:, k, :], A32[:, k, :])
                # A^T blocks of this chunk -> one PSUM bank -> one contiguous
                # fp8 copy on ACT (the only engine with fast fp8 stores)
                tp = wps.tile([128, CT * 128], BF16, name="tp", tag="w")
                for ci in range(CT):
                    nc.tensor.transpose(
                        tp[:, ci * 128:(ci + 1) * 128],
                        A16[:, k, ci * 128:(ci + 1) * 128],
                        ident[:],
                    )
                nc.scalar.copy(AT8[:, k, :], tp[:])
                # upper-triangle Gram matmuls for this chunk (bf16)
                for mi in range(CT):
                    nc.tensor.matmul(
                        g_ps[mi],
                        A16[:, k, mi * 128:(mi + 1) * 128],
                        A16[:, k, mi * 128:],
                        start=(k == 0),
                        stop=(k == NT - 1),
                        # g1/g3 share a bank; per-element has_written makes
                        # disjoint-region groups safe on HW
                        skip_group_check=(mi % 2 == 1),
                    )
            k0 += gsz

        # G (upper) PSUM -> SBUF
        for mi in range(CT):
            if mi % 2 == 0:
                nc.vector.tensor_copy(G32[:, mi, mi * 128:], g_ps[mi])
            else:
                nc.scalar.copy(G32[:, mi, mi * 128:], g_ps[mi])
        # reconstruct lower triangle: G[mi, j] = G[j, mi]^T for j < mi
        for mi in range(1, CT):
            for j in range(mi):
                lb = wps.tile([128, 128], F32, name="lb", tag="w")
                nc.tensor.transpose(
                    lb[:], G32[:, j, mi * 128:(mi + 1) * 128], ident32[:])
                if (mi + j) % 2 == 0:
                    nc.vector.tensor_copy(G32[:, mi, j * 128:(j + 1) * 128], lb[:])
                else:
                    nc.scalar.copy(G32[:, mi, j * 128:(j + 1) * 128], lb[:])

        # softmax over rows of G (free axis); P8 needs an ACT pass (the only
        # engine with fast fp8 stores), interleaved chunk by chunk
        for mi in range(CT):
            nmax = stat.tile([128, 1], F32)
            nc.vector.tensor_reduce(
                nmax[:], G32[:, mi, :],
                axis=mybir.AxisListType.X, op=mybir.AluOpType.max, negate=True,
            )
            esum = stat.tile([128, 1], F32)
            nc.scalar.activation(
                E16[:, mi, :], G32[:, mi, :],
                mybir.ActivationFunctionType.Exp,
                bias=nmax[:], scale=1.0, accum_out=esum[:],
            )
            rsum = stat.tile([128, 1], F32)
            nc.vector.reciprocal(rsum[:], esum[:])
            nc.vector.tensor_scalar_mul(P16[:, mi, :], E16[:, mi, :], rsum[:])
            nc.scalar.copy(P8[:, mi, :], P16[:, mi, :])

        # Y = A @ P (fp8 DoubleRow), epilogue out = gamma * Y + x (f32, exact)
        out_groups = [OGRP] * (NT // OGRP - 1) + [1, 1]
        t0 = 0
        for h, osz in enumerate(out_groups):
            r0 = t0 * 128
            r1 = (t0 + osz) * 128
            o32 = ostage.tile([128, OGRP, C], F32)
            for j in range(osz):
                t = t0 + j
                y = wps.tile([128, C], F32, name="y", tag="w")
                for cp in range(CT // 2):
                    nc.tensor.matmul(
                        y[:],
                        AT8[:, t, 2 * cp * 128:(2 * cp + 2) * 128].rearrange(
                            "p (ko n) -> p ko n", ko=2),
                        P8[:, 2 * cp:2 * cp + 2, :],
                        start=(cp == 0),
                        stop=(cp == CT // 2 - 1),
                        perf_mode=DR,
                    )
                nc.vector.scalar_tensor_tensor(
                    o32[:, j, :], y[:], gB[:], A32[:, t, :],
                    op0=mybir.AluOpType.mult, op1=mybir.AluOpType.add,
                )
            # alternate output rings: ACT HWDGE and (now idle) Sync HWDGE
            oeng = nc.scalar if h % 2 == 0 else nc.sync
            oeng.dma_start(
                out[r0:r1, :].rearrange("(t p) c -> p t c", p=128),
                o32[:, 0:osz, :],
            )
            t0 += osz


def build():
    nc = bacc.Bacc("TRN2", target_bir_lowering=False, debug=False)
    x = nc.dram_tensor("x", [HW, C], F32, kind="ExternalInput").ap()
    gamma = nc.dram_tensor("gamma", [128, 1], F32, kind="ExternalInput").ap()
    out = nc.dram_tensor("out", [HW, C], F32, kind="ExternalOutput").ap()
    with tile.TileContext(nc) as tc:
        _emit(nc, tc, out, x, gamma)
    nc.compile()
    return nc


def kernel(x: np.ndarray, gamma: np.ndarray, trace: bool = False):
    assert x.shape == (B, H, W, C), x.shape
    if "nc" not in _CACHE:
        _CACHE["nc"] = build()
    nc = _CACHE["nc"]

    g128 = np.full((128, 1), np.float32(np.asarray(gamma).reshape(-1)[0]),
                   dtype=np.float32)
    in_maps = [
        {
            "x": np.ascontiguousarray(
                np.asarray(x[i], dtype=np.float32).reshape(HW, C)),
            "gamma": g128,
        }
        for i in range(B)
    ]
    if trace:
        res = run_bass_kernel_spmd(nc, in_maps, core_ids=list(range(B)),
                                   trace=True)
    else:
        # Force-untraced: a stray BASS_TRACE in the environment would route
        # through profiling hooks this image may not have.
        import os
        prev = os.environ.get("BASS_NEVER_TRACE")
        os.environ["BASS_NEVER_TRACE"] = "1"
        try:
            res = run_bass_kernel_spmd(nc, in_maps, core_ids=list(range(B)))
        finally:
            if prev is None:
                os.environ.pop("BASS_NEVER_TRACE", None)
            else:
                os.environ["BASS_NEVER_TRACE"] = prev
    _CACHE["last_result"] = res
    out = np.stack([res.results[i]["out"] for i in range(B)], axis=0)
    return out.reshape(B, H, W, C).astype(np.float32)
